# revision 1
# baseline (speedup 1.0000x reference)
"""SpMM (COO adjacency @ dense weight) on 8 Trainium2 NeuronCores.

out[r] = sum over edges (r, c) of weight[c]   (adj values are all ones)

Strategy (see sharding hint): partition edges by destination row across the
8 cores; replicate the weight table in each core's DRAM. On the host,
output rows are packed into 8*T bins of <=128 rows AND <=128 incoming
edges each (capacity-aware best-fit; in-degrees are small Poisson counts,
so exact 128-slot packing nearly always succeeds — a chunked fallback
covers the rest). Device work per 128-row output tile:
  1. one indirect DMA gathers the tile's 128 source rows of weight
     (slots pre-sorted by source column for HBM row-buffer locality),
  2. the Vector engine builds a 0/1 selection matrix S[e, r] =
     (dest[e] == r) from preloaded per-slot local-dest indices vs iota,
  3. TensorEngine matmul segment-sums psum[r, :] += S^T @ gathered,
  4. PSUM -> SBUF copy (alternating Vector/Scalar engines) and a DMA of
     the [128, 256] f32 output tile to DRAM.
Host then inverse-permutes the concatenated per-core outputs.
"""

import heapq

import numpy as np

NC_CORES = 8
P = 128
# NOTE: the runtime indirect-DMA ucode only honors [P, 1] offset APs (one
# offset per partition); multi-column offsets silently gather garbage on HW
# even though CoreSim accepts them. Keep one indirect DMA per tile chunk.
GROUP = 1  # tiles per batched indirect gather


def _build_program(n_tbl, d, t_tiles, chunks, group=GROUP):
    """Build the SPMD Bass program (identical across cores; data differs).

    chunks: per-tile slot-chunk sizes, e.g. [128] (sum = E slots/tile).
    """
    from contextlib import ExitStack

    import concourse.bacc as bacc
    import concourse.bass as bass
    import concourse.mybir as mybir
    import concourse.tile as tile

    dt = mybir.dt
    nc = bacc.Bacc(None)

    wt = nc.declare_dram_parameter("wt", [n_tbl, d], dt.float32, isOutput=False)
    colp = []
    destp = []
    for ci, cb in enumerate(chunks):
        colp.append(
            nc.declare_dram_parameter(f"cols{ci}", [cb, t_tiles], dt.int32, isOutput=False)
        )
        destp.append(
            nc.declare_dram_parameter(
                f"dest{ci}", [cb, t_tiles], dt.float32, isOutput=False
            )
        )
    iota_p = nc.declare_dram_parameter("iota", [P, P], dt.float32, isOutput=False)
    out_p = nc.declare_dram_parameter("out", [t_tiles * P, d], dt.float32, isOutput=True)

    with tile.TileContext(nc) as tc:
        with ExitStack() as ctx:
            cpool = ctx.enter_context(tc.tile_pool(name="const", bufs=1))
            gpools = [
                ctx.enter_context(tc.tile_pool(name=f"g{ci}", bufs=10))
                for ci in range(len(chunks))
            ]
            spools = [
                ctx.enter_context(tc.tile_pool(name=f"s{ci}", bufs=10))
                for ci in range(len(chunks))
            ]
            opool = ctx.enter_context(tc.tile_pool(name="o", bufs=10))
            pspool = ctx.enter_context(tc.tile_pool(name="ps", bufs=8, space="PSUM"))

            iota_sb = cpool.tile([P, P], dtype=dt.float32)
            nc.sync.dma_start(iota_sb[:], iota_p[:])
            cols_sb = []
            dest_sb = []
            for ci, cb in enumerate(chunks):
                ct = cpool.tile([cb, t_tiles], dtype=dt.int32, tag=f"cols{ci}")
                nc.sync.dma_start(ct[:], colp[ci][:])
                cols_sb.append(ct)
                dtile = cpool.tile([cb, t_tiles], dtype=dt.float32, tag=f"dest{ci}")
                nc.sync.dma_start(dtile[:], destp[ci][:])
                dest_sb.append(dtile)

            for g0 in range(0, t_tiles, group):
                kk = min(group, t_tiles - g0)
                gs = []
                for ci, cb in enumerate(chunks):
                    # 2-D out AP when kk == 1: the runtime indirect-DMA
                    # ucode mishandles a trailing unit middle dim.
                    shape = [cb, d] if kk == 1 else [cb, kk, d]
                    gt = gpools[ci].tile(shape, dtype=dt.float32, tag=f"g{ci}")
                    nc.gpsimd.indirect_dma_start(
                        out=gt[:],
                        out_offset=None,
                        in_=wt[:],
                        in_offset=bass.IndirectOffsetOnAxis(
                            ap=cols_sb[ci][:, g0 : g0 + kk], axis=0
                        ),
                    )
                    gs.append(gt)
                for j in range(kk):
                    t = g0 + j
                    ps = pspool.tile([P, d], dtype=dt.float32)
                    for ci, cb in enumerate(chunks):
                        s = spools[ci].tile([cb, P], dtype=dt.float32, tag=f"s{ci}")
                        nc.vector.tensor_tensor(
                            out=s[:],
                            in0=dest_sb[ci][:, t : t + 1].to_broadcast([cb, P]),
                            in1=iota_sb[:cb, :],
                            op=mybir.AluOpType.is_equal,
                        )
                        rhs = gs[ci][:] if kk == 1 else gs[ci][:, j, :]
                        nc.tensor.matmul(
                            out=ps[:],
                            lhsT=s[:],
                            rhs=rhs,
                            start=(ci == 0),
                            stop=(ci == len(chunks) - 1),
                        )
                    ot = opool.tile([P, d], dtype=dt.float32)
                    if t % 2 == 0:
                        nc.vector.tensor_copy(out=ot[:], in_=ps[:])
                    else:
                        nc.scalar.copy(out=ot[:], in_=ps[:])
                    nc.sync.dma_start(out_p[t * P : (t + 1) * P, :], ot[:])

    nc.finalize()
    return nc


def _pack_bins_exact(rows, counts, nbins):
    """Best-fit pack rows into bins with <=128 slots AND <=128 rows each.

    Returns (bin_of_row, pos_of_row) or None if infeasible.
    """
    n = len(counts)
    if nbins * P < counts.sum() or counts.max() > P:
        return None
    nz = np.flatnonzero(counts)
    order = nz[np.argsort(-counts[nz], kind="stable")]
    bin_of_row = np.full(n, -1, np.int64)
    loads = np.zeros(nbins, np.int64)
    nrows = np.zeros(nbins, np.int64)
    heap = [(0, b) for b in range(nbins)]
    heapq.heapify(heap)
    for r in order.tolist():
        c = int(counts[r])
        while True:
            if not heap:
                return None
            load, b = heapq.heappop(heap)
            if load != loads[b] or nrows[b] >= P:
                continue  # stale entry or row-capacity full
            break
        if load + c > P:
            return None  # min-load bin can't fit -> nothing can
        bin_of_row[r] = b
        loads[b] += c
        nrows[b] += 1
        if loads[b] < P and nrows[b] < P:
            heapq.heappush(heap, (int(loads[b]), b))
    # zero-count rows fill the remaining row capacity anywhere
    zeros = np.flatnonzero(counts == 0)
    cap = P - nrows
    if cap.sum() < len(zeros):
        return None
    fill_bins = np.repeat(np.arange(nbins), cap)[: len(zeros)]
    bin_of_row[zeros] = fill_bins
    # positions: stable order within bin
    order_all = np.argsort(bin_of_row, kind="stable")
    bins_sorted = bin_of_row[order_all]
    starts = np.searchsorted(bins_sorted, np.arange(nbins))
    pos_of_row = np.empty(n, np.int64)
    pos_of_row[order_all] = np.arange(n, dtype=np.int64) - starts[bins_sorted]
    if pos_of_row.max() >= P:
        return None
    return bin_of_row, pos_of_row


def _pack_bins_dealt(counts, nbins):
    """Fallback: deal count-sorted rows round-robin (E may exceed 128)."""
    n = len(counts)
    order = np.argsort(-counts, kind="stable")
    idx = np.arange(n, dtype=np.int64)
    bin_of_row = np.empty(n, np.int64)
    pos_of_row = np.empty(n, np.int64)
    bin_of_row[order] = idx % nbins
    pos_of_row[order] = idx // nbins
    return bin_of_row, pos_of_row


def _prepare(adj, weight):
    """Host-side sharding: pack rows into bins, pack edges into slots."""
    w = np.ascontiguousarray(np.asarray(weight, dtype=np.float32))
    n, d = w.shape
    adj = np.asarray(adj)
    rows = adj[0].astype(np.int64)
    cols = adj[1].astype(np.int64)

    t_tiles = -(-n // (NC_CORES * P))  # ceil
    nbins = NC_CORES * t_tiles

    counts = np.bincount(rows, minlength=n)
    packed = _pack_bins_exact(rows, counts, nbins)
    if packed is None:
        packed = _pack_bins_dealt(counts, nbins)
    bin_of_row, pos_of_row = packed
    assert pos_of_row.max() < P

    # Slot assignment: edges of a bin occupy consecutive slots, ordered by
    # ascending source column — the gather's HBM reads then walk ascending
    # addresses (better row-buffer locality). Slot order within a bin is
    # free: the selection matrix maps any slot to its output row.
    eb = bin_of_row[rows]
    eo = np.lexsort((cols, eb))
    sb = eb[eo]
    starts = np.searchsorted(sb, np.arange(nbins))
    slot = np.arange(len(eo), dtype=np.int64) - starts[sb]

    e_slots = int(np.bincount(eb, minlength=nbins).max())
    if e_slots <= P:
        e_slots = P
    else:
        e_slots = max(e_slots, P + 4)
        e_slots = -4 * (-e_slots // 4)  # round up to multiple of 4

    cols_full = np.zeros((nbins, e_slots), np.int32)  # pad -> gather row 0
    dest_full = np.full((nbins, e_slots), -1.0, np.float32)  # pad: no match
    cols_full[sb, slot] = cols[eo].astype(np.int32)
    dest_full[sb, slot] = pos_of_row[rows[eo]].astype(np.float32)

    chunks = []
    rem = e_slots
    while rem > 0:
        c = min(rem, P)
        chunks.append(c)
        rem -= c

    iota = np.ascontiguousarray(
        np.broadcast_to(np.arange(P, dtype=np.float32), (P, P))
    )
    in_maps = []
    for c in range(NC_CORES):
        b0 = c * t_tiles
        b1 = b0 + t_tiles
        m = {"wt": w, "iota": iota}
        off = 0
        for ci, cb in enumerate(chunks):
            m[f"cols{ci}"] = np.ascontiguousarray(cols_full[b0:b1, off : off + cb].T)
            m[f"dest{ci}"] = np.ascontiguousarray(dest_full[b0:b1, off : off + cb].T)
            off += cb
        in_maps.append(m)

    meta = {
        "n": n,
        "d": d,
        "t_tiles": t_tiles,
        "chunks": chunks,
        "bin_of_row": bin_of_row,
        "pos_of_row": pos_of_row,
    }
    return in_maps, meta


LAST_RESULT = None


def kernel(adj, size, weight):
    global LAST_RESULT
    from concourse.bass_utils import run_bass_kernel_spmd

    in_maps, meta = _prepare(adj, weight)
    nc = _build_program(meta["n"], meta["d"], meta["t_tiles"], meta["chunks"])
    res = run_bass_kernel_spmd(nc, in_maps, core_ids=list(range(NC_CORES)))
    LAST_RESULT = res
    flat = np.concatenate([r["out"] for r in res.results], axis=0)
    return flat[meta["bin_of_row"] * P + meta["pos_of_row"]]



# revision 3
# speedup vs baseline: 1.1700x; 1.1700x over previous
"""SpMM (COO adjacency @ dense weight) on 8 Trainium2 NeuronCores.

out[r] = sum over edges (r, c) of weight[c]   (adj values are all ones)

Strategy: partition edges by destination row across the 8 cores (see
sharding hint). Host packs output rows into 8*T bins of <=128 rows AND
<=128 incoming edges each (capacity-aware best-fit over T=100 tiles/core,
which leaves ~2.4% slot slack so exact packing succeeds). Per core the
host builds a compacted bf16 weight table holding only that core's
unique source columns (~11.8K rows, so slot indices fit int16) plus an
int16 slot->table-row index list and a per-slot local-dest array.

Device work per core (all bf16 data path, fp32 PSUM accumulate):
  1. dma_gather (MoE-style SWDGE gather ucode) pulls CHUNK*128 table
     rows per instruction into SBUF as [128, CHUNK, 256] — slot i lands
     at partition i%128, free slot i//128, exactly the matmul rhs
     layout. ~10 gather instructions replace the baseline's 98
     serialized indirect DMAs (descriptor-gen cost 994ns + 0.34ns/row).
  2. Per tile, Vector builds the 0/1 selection matrix
     S[e, r] = (dest[e] == r) in bf16; TensorEngine matmul
     psum[r, :] += S^T @ gathered does the segment-sum (bf16 matmul is
     4x the fp32 rate).
  3. PSUM -> SBUF bf16 copies (alternating Vector/Scalar engines),
     staged CHUNK tiles at a time, then one 640KB HWDGE write per chunk
     to a partition-major [128, T, 256] bf16 output tensor.
Host inverse-permutes the per-core outputs and upcasts to f32 (bf16
rounding of in/out is ~0.4% worst case vs the 2e-2 tolerance).
"""

import heapq

import ml_dtypes
import numpy as np

NC_CORES = 8
P = 128
T_TILES = 100  # output tiles (bins) per core; 8*T*128 row slots total
CHUNK = 10  # tiles per dma_gather / per output write


def _build_program(n_tbl, d, t_tiles, chunk):
    """Build the SPMD Bass program (identical across cores; data differs)."""
    from contextlib import ExitStack

    import concourse.bacc as bacc
    import concourse.mybir as mybir
    import concourse.tile as tile

    dt = mybir.dt
    nc = bacc.Bacc(None)

    idx_cols = (t_tiles * P) // 16
    wt = nc.declare_dram_parameter("wt", [n_tbl, d], dt.bfloat16, isOutput=False)
    idx_p = nc.declare_dram_parameter("idx", [P, idx_cols], dt.int16, isOutput=False)
    dest_p = nc.declare_dram_parameter("dest", [P, t_tiles], dt.float32, isOutput=False)
    iota_p = nc.declare_dram_parameter("iota", [P, P], dt.float32, isOutput=False)
    out_p = nc.declare_dram_parameter(
        "out", [P, t_tiles, d], dt.bfloat16, isOutput=True
    )

    with tile.TileContext(nc) as tc:
        with ExitStack() as ctx:
            cpool = ctx.enter_context(tc.tile_pool(name="const", bufs=1))
            gpool = ctx.enter_context(tc.tile_pool(name="g", bufs=3))
            spool = ctx.enter_context(tc.tile_pool(name="s", bufs=8))
            opool = ctx.enter_context(tc.tile_pool(name="o", bufs=3))
            pspool = ctx.enter_context(tc.tile_pool(name="ps", bufs=8, space="PSUM"))

            iota_sb = cpool.tile([P, P], dtype=dt.float32)
            nc.sync.dma_start(iota_sb[:], iota_p[:])
            dest_sb = cpool.tile([P, t_tiles], dtype=dt.float32)
            nc.sync.dma_start(dest_sb[:], dest_p[:])
            idx_sb = cpool.tile([P, idx_cols], dtype=dt.int16)
            nc.sync.dma_start(idx_sb[:], idx_p[:])

            for g0 in range(0, t_tiles, chunk):
                k = min(chunk, t_tiles - g0)
                gt = gpool.tile([P, k, d], dtype=dt.bfloat16, tag="g")
                nc.gpsimd.dma_gather(
                    out_ap=gt[:],
                    in_ap=wt[:],
                    idxs_ap=idx_sb[:, g0 * 8 : (g0 + k) * 8],
                    num_idxs=P * k,
                    num_idxs_reg=P * k,
                    elem_size=d,
                    # >64 descriptors per SDMA engine don't fit one packet;
                    # the packed-packet ucode path DMA-aborts above 1024 idxs.
                    single_packet=False,
                )
                ot = opool.tile([P, k, d], dtype=dt.bfloat16, tag="o")
                for j in range(k):
                    t = g0 + j
                    s = spool.tile([P, P], dtype=dt.bfloat16, tag="s")
                    nc.vector.tensor_tensor(
                        out=s[:],
                        in0=dest_sb[:, t : t + 1].to_broadcast([P, P]),
                        in1=iota_sb[:],
                        op=mybir.AluOpType.is_equal,
                    )
                    ps = pspool.tile([P, d], dtype=dt.float32)
                    nc.tensor.matmul(
                        out=ps[:], lhsT=s[:], rhs=gt[:, j, :], start=True, stop=True
                    )
                    if t % 2 == 0:
                        nc.vector.tensor_copy(out=ot[:, j, :], in_=ps[:])
                    else:
                        nc.scalar.copy(out=ot[:, j, :], in_=ps[:])
                nc.sync.dma_start(out_p[:, g0 : g0 + k, :], ot[:])

    nc.finalize()
    return nc


def _pack_bins_exact(rows, counts, nbins):
    """Best-fit pack rows into bins with <=128 slots AND <=128 rows each.

    Returns (bin_of_row, pos_of_row) or None if infeasible.
    """
    n = len(counts)
    if nbins * P < counts.sum() or counts.max() > P:
        return None
    nz = np.flatnonzero(counts)
    order = nz[np.argsort(-counts[nz], kind="stable")]
    bin_of_row = np.full(n, -1, np.int64)
    loads = np.zeros(nbins, np.int64)
    nrows = np.zeros(nbins, np.int64)
    heap = [(0, b) for b in range(nbins)]
    heapq.heapify(heap)
    for r in order.tolist():
        c = int(counts[r])
        while True:
            if not heap:
                return None
            load, b = heapq.heappop(heap)
            if load != loads[b] or nrows[b] >= P:
                continue  # stale entry or row-capacity full
            break
        if load + c > P:
            return None  # min-load bin can't fit -> nothing can
        bin_of_row[r] = b
        loads[b] += c
        nrows[b] += 1
        if loads[b] < P and nrows[b] < P:
            heapq.heappush(heap, (int(loads[b]), b))
    # zero-count rows fill the remaining row capacity anywhere
    zeros = np.flatnonzero(counts == 0)
    cap = P - nrows
    if cap.sum() < len(zeros):
        return None
    fill_bins = np.repeat(np.arange(nbins), cap)[: len(zeros)]
    bin_of_row[zeros] = fill_bins
    # positions: stable order within bin
    order_all = np.argsort(bin_of_row, kind="stable")
    bins_sorted = bin_of_row[order_all]
    starts = np.searchsorted(bins_sorted, np.arange(nbins))
    pos_of_row = np.empty(n, np.int64)
    pos_of_row[order_all] = np.arange(n, dtype=np.int64) - starts[bins_sorted]
    if pos_of_row.max() >= P:
        return None
    return bin_of_row, pos_of_row


def _prepare(adj, weight):
    """Host-side sharding: pack rows into bins, build per-core gather data."""
    w = np.ascontiguousarray(np.asarray(weight, dtype=np.float32))
    n, d = w.shape
    adj = np.asarray(adj)
    rows = adj[0].astype(np.int64)
    cols = adj[1].astype(np.int64)

    counts = np.bincount(rows, minlength=n)
    t_tiles = T_TILES
    while True:
        nbins = NC_CORES * t_tiles
        packed = _pack_bins_exact(rows, counts, nbins)
        if packed is not None:
            break
        t_tiles += 2  # more slack; terminates long before degree bound bites
    bin_of_row, pos_of_row = packed

    # Edge slots: edges of a bin occupy consecutive slots ordered by source
    # column (ascending table reads within each tile chunk).
    eb = bin_of_row[rows]
    eo = np.lexsort((cols, eb))
    sb = eb[eo]
    starts = np.searchsorted(sb, np.arange(nbins))
    slot_in_bin = np.arange(len(eo), dtype=np.int64) - starts[sb]

    w_bf = w.astype(ml_dtypes.bfloat16)
    slots = t_tiles * P

    per_core = []
    for c in range(NC_CORES):
        sel = (sb // t_tiles) == c
        cols_c = cols[eo[sel]]
        gslot = (sb[sel] % t_tiles) * P + slot_in_bin[sel]
        uniq, inv = np.unique(cols_c, return_inverse=True)
        assert len(uniq) < 32768, len(uniq)
        idx_flat = np.zeros(slots, np.int16)
        dest_flat = np.full(slots, -1.0, np.float32)
        idx_flat[gslot] = inv.astype(np.int16)
        dest_flat[gslot] = pos_of_row[rows[eo[sel]]].astype(np.float32)
        per_core.append((uniq, idx_flat, dest_flat))

    n_tbl = max(len(u) for u, _, _ in per_core)
    n_tbl = -P * (-n_tbl // P)  # round up to multiple of 128

    iota = np.ascontiguousarray(
        np.broadcast_to(np.arange(P, dtype=np.float32), (P, P))
    )
    in_maps = []
    for uniq, idx_flat, dest_flat in per_core:
        tbl = np.zeros((n_tbl, d), ml_dtypes.bfloat16)
        tbl[: len(uniq)] = w_bf[uniq]
        idx_wrapped = np.ascontiguousarray(idx_flat.reshape(-1, 16).T)  # [16, slots/16]
        idx_full = np.ascontiguousarray(np.tile(idx_wrapped, (8, 1)))  # [128, ...]
        dest_arr = np.ascontiguousarray(dest_flat.reshape(t_tiles, P).T)  # [128, T]
        in_maps.append({"wt": tbl, "idx": idx_full, "dest": dest_arr, "iota": iota})

    meta = {
        "n": n,
        "d": d,
        "t_tiles": t_tiles,
        "n_tbl": n_tbl,
        "bin_of_row": bin_of_row,
        "pos_of_row": pos_of_row,
    }
    return in_maps, meta


LAST_RESULT = None


def kernel(adj, size, weight):
    global LAST_RESULT
    from concourse.bass_utils import run_bass_kernel_spmd

    in_maps, meta = _prepare(adj, weight)
    nc = _build_program(meta["n_tbl"], meta["d"], meta["t_tiles"], CHUNK)
    res = run_bass_kernel_spmd(nc, in_maps, core_ids=list(range(NC_CORES)))
    LAST_RESULT = res
    t_tiles = meta["t_tiles"]
    # stack: [core, 128, T, d] -> index rows by (core, pos, local_tile)
    big = np.stack([np.asarray(r["out"]) for r in res.results])
    b = meta["bin_of_row"]
    out = big[b // t_tiles, meta["pos_of_row"], b % t_tiles, :]
    return np.ascontiguousarray(out.astype(np.float32))


# revision 11
# speedup vs baseline: 2.5942x; 2.2172x over previous
"""SpMM (COO adjacency @ dense weight) on 8 Trainium2 NeuronCores.

out[r] = sum over edges (r, c) of weight[c]   (adj values are all ones)

Strategy: partition edges by destination row across the 8 cores (see
sharding hint). Host packs output rows into 8*T bins of <=128 rows AND
<=128 incoming edges each (capacity-aware best-fit over T=100 tiles/core,
which leaves ~2.4% slot slack so exact packing succeeds). Per core the
host builds a compacted bf16 weight table holding only that core's
unique source columns (~11.8K rows, so slot indices fit int16) plus an
int16 slot->table-row index list and a per-slot local-dest array.

Device work per core (all bf16 data path, fp32 PSUM accumulate):
  1. dma_gather (MoE-style SWDGE gather ucode) pulls CHUNK*128 table
     rows per instruction into SBUF as [128, CHUNK, 256] — slot i lands
     at partition i%128, free slot i//128, exactly the matmul rhs
     layout. ~10 gather instructions replace the baseline's 98
     serialized indirect DMAs (descriptor-gen cost 994ns + 0.34ns/row).
  2. Per tile, Vector builds the 0/1 selection matrix
     S[e, r] = (dest[e] == r) in bf16; TensorEngine matmul
     psum[r, :] += S^T @ gathered does the segment-sum (bf16 matmul is
     4x the fp32 rate).
  3. PSUM -> SBUF bf16 copies (alternating Vector/Scalar engines),
     staged CHUNK tiles at a time, then one 640KB HWDGE write per chunk
     to a partition-major [128, T, 256] bf16 output tensor.
Host inverse-permutes the per-core outputs and upcasts to f32 (bf16
rounding of in/out is ~0.4% worst case vs the 2e-2 tolerance).
"""

import heapq
import os

import ml_dtypes
import numpy as np

NC_CORES = 8
P = 128
T_TILES = 100  # output tiles (bins) per core; 8*T*128 row slots total
CHUNK = 10  # tiles per dma_gather / per output write
# "gather": device-side dma_gather from a per-core unique-column table.
# "stream": host lays per-slot rows out in gather-result order; device
#           streams them with bulk HWDGE DMAs (no SWDGE desc-gen).
MODE = os.environ.get("KMODE", "gather")


def _build_program(n_tbl, d, t_tiles, chunk, mode=None):
    """Build the SPMD Bass program (identical across cores; data differs)."""
    from contextlib import ExitStack

    import concourse.bacc as bacc
    import concourse.mybir as mybir
    import concourse.tile as tile

    mode = mode or MODE
    dt = mybir.dt
    nc = bacc.Bacc(None, num_swdge_queues=4)

    idx_cols = (t_tiles * P) // 16
    if mode == "gather":
        wt = nc.declare_dram_parameter("wt", [n_tbl, d], dt.bfloat16, isOutput=False)
        idx_p = nc.declare_dram_parameter(
            "idx", [P, idx_cols], dt.int16, isOutput=False
        )
    else:
        wt = nc.declare_dram_parameter(
            "wt", [P, t_tiles, d], dt.bfloat16, isOutput=False
        )
    dest_p = nc.declare_dram_parameter("dest", [P, t_tiles], dt.float32, isOutput=False)
    iota_p = nc.declare_dram_parameter("iota", [P, P], dt.float32, isOutput=False)
    out_p = nc.declare_dram_parameter(
        "out", [P, t_tiles, d], dt.bfloat16, isOutput=True
    )

    with tile.TileContext(nc) as tc:
        with ExitStack() as ctx:
            cpool = ctx.enter_context(tc.tile_pool(name="const", bufs=1))
            gpool = ctx.enter_context(tc.tile_pool(name="g", bufs=5))
            spool = ctx.enter_context(tc.tile_pool(name="s", bufs=8))
            opool = ctx.enter_context(tc.tile_pool(name="o", bufs=3))
            pspool = ctx.enter_context(tc.tile_pool(name="ps", bufs=8, space="PSUM"))

            if mode == "gather":
                idx_sb = cpool.tile([P, idx_cols], dtype=dt.int16)
                nc.sync.dma_start(idx_sb[:], idx_p[:])
            iota_sb = cpool.tile([P, P], dtype=dt.float32)
            nc.sync.dma_start(iota_sb[:], iota_p[:])
            dest_sb = cpool.tile([P, t_tiles], dtype=dt.float32)
            nc.sync.dma_start(dest_sb[:], dest_p[:])

            for gi, g0 in enumerate(range(0, t_tiles, chunk)):
                k = min(chunk, t_tiles - g0)
                gt = gpool.tile([P, k, d], dtype=dt.bfloat16, tag="g")
                if mode == "gather":
                    nc.gpsimd.dma_gather(
                        out_ap=gt[:],
                        in_ap=wt[:],
                        idxs_ap=idx_sb[:, g0 * 8 : (g0 + k) * 8],
                        num_idxs=P * k,
                        num_idxs_reg=P * k,
                        elem_size=d,
                        # >64 descriptors per SDMA engine don't fit one
                        # packet; the packed-packet path DMA-aborts >1024.
                        single_packet=False,
                        # queue 0 desc-gen occupies the Pool engine; 1-3
                        # run concurrently off-engine. Dispatch async queues
                        # first so q0's on-engine gen doesn't delay them.
                        queue_num=(1, 2, 3, 0)[gi % 4],
                    )
                else:
                    nc.sync.dma_start(gt[:], wt[:, g0 : g0 + k, :])
                ot = opool.tile([P, k, d], dtype=dt.bfloat16, tag="o")
                for j in range(k):
                    t = g0 + j
                    s = spool.tile([P, P], dtype=dt.bfloat16, tag="s")
                    nc.vector.tensor_tensor(
                        out=s[:],
                        in0=dest_sb[:, t : t + 1].to_broadcast([P, P]),
                        in1=iota_sb[:],
                        op=mybir.AluOpType.is_equal,
                    )
                    ps = pspool.tile([P, d], dtype=dt.float32)
                    nc.tensor.matmul(
                        out=ps[:], lhsT=s[:], rhs=gt[:, j, :], start=True, stop=True
                    )
                    if t % 2 == 0:
                        nc.vector.tensor_copy(out=ot[:, j, :], in_=ps[:])
                    else:
                        nc.scalar.copy(out=ot[:, j, :], in_=ps[:])
                nc.sync.dma_start(out_p[:, g0 : g0 + k, :], ot[:])

    nc.finalize()
    return nc


def _pack_bins_exact(rows, counts, nbins):
    """Best-fit pack rows into bins with <=128 slots AND <=128 rows each.

    Returns (bin_of_row, pos_of_row) or None if infeasible.
    """
    n = len(counts)
    if nbins * P < counts.sum() or counts.max() > P:
        return None
    nz = np.flatnonzero(counts)
    order = nz[np.argsort(-counts[nz], kind="stable")]
    bin_of_row = np.full(n, -1, np.int64)
    loads = np.zeros(nbins, np.int64)
    nrows = np.zeros(nbins, np.int64)
    heap = [(0, b) for b in range(nbins)]
    heapq.heapify(heap)
    for r in order.tolist():
        c = int(counts[r])
        while True:
            if not heap:
                return None
            load, b = heapq.heappop(heap)
            if load != loads[b] or nrows[b] >= P:
                continue  # stale entry or row-capacity full
            break
        if load + c > P:
            return None  # min-load bin can't fit -> nothing can
        bin_of_row[r] = b
        loads[b] += c
        nrows[b] += 1
        if loads[b] < P and nrows[b] < P:
            heapq.heappush(heap, (int(loads[b]), b))
    # zero-count rows fill the remaining row capacity anywhere
    zeros = np.flatnonzero(counts == 0)
    cap = P - nrows
    if cap.sum() < len(zeros):
        return None
    fill_bins = np.repeat(np.arange(nbins), cap)[: len(zeros)]
    bin_of_row[zeros] = fill_bins
    # positions: stable order within bin
    order_all = np.argsort(bin_of_row, kind="stable")
    bins_sorted = bin_of_row[order_all]
    starts = np.searchsorted(bins_sorted, np.arange(nbins))
    pos_of_row = np.empty(n, np.int64)
    pos_of_row[order_all] = np.arange(n, dtype=np.int64) - starts[bins_sorted]
    if pos_of_row.max() >= P:
        return None
    return bin_of_row, pos_of_row


def _prepare(adj, weight):
    """Host-side sharding: pack rows into bins, build per-core gather data."""
    w = np.ascontiguousarray(np.asarray(weight, dtype=np.float32))
    n, d = w.shape
    adj = np.asarray(adj)
    rows = adj[0].astype(np.int64)
    cols = adj[1].astype(np.int64)

    counts = np.bincount(rows, minlength=n)
    t_tiles = T_TILES
    while True:
        nbins = NC_CORES * t_tiles
        packed = _pack_bins_exact(rows, counts, nbins)
        if packed is not None:
            break
        t_tiles += 2  # more slack; terminates long before degree bound bites
    bin_of_row, pos_of_row = packed

    # Edge slots: edges of a bin occupy consecutive slots ordered by source
    # column (ascending table reads within each tile chunk).
    eb = bin_of_row[rows]
    eo = np.lexsort((cols, eb))
    sb = eb[eo]
    starts = np.searchsorted(sb, np.arange(nbins))
    slot_in_bin = np.arange(len(eo), dtype=np.int64) - starts[sb]

    w_bf = w.astype(ml_dtypes.bfloat16)
    slots = t_tiles * P

    per_core = []
    for c in range(NC_CORES):
        sel = (sb // t_tiles) == c
        cols_c = cols[eo[sel]]
        gslot = (sb[sel] % t_tiles) * P + slot_in_bin[sel]
        uniq, inv = np.unique(cols_c, return_inverse=True)
        assert len(uniq) < 32768, len(uniq)
        idx_flat = np.zeros(slots, np.int16)
        dest_flat = np.full(slots, -1.0, np.float32)
        idx_flat[gslot] = inv.astype(np.int16)
        dest_flat[gslot] = pos_of_row[rows[eo[sel]]].astype(np.float32)
        col_flat = np.zeros(slots, np.int64)
        col_flat[gslot] = cols_c
        per_core.append((uniq, idx_flat, dest_flat, col_flat))

    n_tbl = max(len(u) for u, _, _, _ in per_core)
    n_tbl = -P * (-n_tbl // P)  # round up to multiple of 128

    iota = np.ascontiguousarray(
        np.broadcast_to(np.arange(P, dtype=np.float32), (P, P))
    )
    in_maps = []
    for uniq, idx_flat, dest_flat, col_flat in per_core:
        if MODE == "gather":
            tbl = np.zeros((n_tbl, d), ml_dtypes.bfloat16)
            tbl[: len(uniq)] = w_bf[uniq]
        else:
            # slot-ordered rows, partition-major: tbl[p, t, :] = row of
            # slot t*128+p (the layout dma_gather would produce).
            tbl = np.ascontiguousarray(
                w_bf[col_flat].reshape(t_tiles, P, d).transpose(1, 0, 2)
            )
        idx_wrapped = np.ascontiguousarray(idx_flat.reshape(-1, 16).T)  # [16, slots/16]
        idx_full = np.ascontiguousarray(np.tile(idx_wrapped, (8, 1)))  # [128, ...]
        dest_arr = np.ascontiguousarray(dest_flat.reshape(t_tiles, P).T)  # [128, T]
        m = {"wt": tbl, "dest": dest_arr, "iota": iota}
        if MODE == "gather":
            m["idx"] = idx_full
        in_maps.append(m)

    meta = {
        "n": n,
        "d": d,
        "t_tiles": t_tiles,
        "n_tbl": n_tbl,
        "bin_of_row": bin_of_row,
        "pos_of_row": pos_of_row,
    }
    return in_maps, meta


LAST_RESULT = None


def kernel(adj, size, weight):
    global LAST_RESULT
    from concourse.bass_utils import run_bass_kernel_spmd

    in_maps, meta = _prepare(adj, weight)
    nc = _build_program(meta["n_tbl"], meta["d"], meta["t_tiles"], CHUNK)
    res = run_bass_kernel_spmd(nc, in_maps, core_ids=list(range(NC_CORES)))
    LAST_RESULT = res
    t_tiles = meta["t_tiles"]
    # stack: [core, 128, T, d] -> index rows by (core, pos, local_tile)
    big = np.stack([np.asarray(r["out"]) for r in res.results])
    b = meta["bin_of_row"]
    out = big[b // t_tiles, meta["pos_of_row"], b % t_tiles, :]
    return np.ascontiguousarray(out.astype(np.float32))


# revision 18
# speedup vs baseline: 2.7193x; 1.0482x over previous
"""SpMM (COO adjacency @ dense weight) on 8 Trainium2 NeuronCores.

out[r] = sum over edges (r, c) of weight[c]   (adj values are all ones)

Strategy: partition edges by destination row across the 8 cores (see
sharding hint). Host packs output rows into 8*T bins of <=128 rows AND
<=128 incoming edges each (capacity-aware best-fit over T=100 tiles/core,
which leaves ~2.4% slot slack so exact packing succeeds). Per core the
host builds a compacted bf16 weight table holding only that core's
unique source columns (~11.8K rows, so slot indices fit int16) plus an
int16 slot->table-row index list and a per-slot local-dest array.

Device work per core (all bf16 data path, fp32 PSUM accumulate):
  1. dma_gather (MoE-style SWDGE gather ucode) pulls CHUNK*128 table
     rows per instruction into SBUF as [128, CHUNK, 256] — slot i lands
     at partition i%128, free slot i//128, exactly the matmul rhs
     layout. ~10 gather instructions replace the baseline's 98
     serialized indirect DMAs (descriptor-gen cost 994ns + 0.34ns/row).
  2. Per tile, Vector builds the 0/1 selection matrix
     S[e, r] = (dest[e] == r) in bf16; TensorEngine matmul
     psum[r, :] += S^T @ gathered does the segment-sum (bf16 matmul is
     4x the fp32 rate).
  3. PSUM -> SBUF bf16 copies (alternating Vector/Scalar engines),
     staged CHUNK tiles at a time, then one 640KB HWDGE write per chunk
     to a partition-major [128, T, 256] bf16 output tensor.
Host inverse-permutes the per-core outputs and upcasts to f32 (bf16
rounding of in/out is ~0.4% worst case vs the 2e-2 tolerance).
"""

import heapq
import os

import ml_dtypes
import numpy as np

NC_CORES = 8
P = 128
T_TILES = 100  # output tiles (bins) per core; 8*T*128 row slots total
CHUNK = 10  # tiles per dma_gather / per output write
# "gather": device-side dma_gather from a per-core unique-column table.
# "stream": host lays per-slot rows out in gather-result order; device
#           streams them with bulk HWDGE DMAs (no SWDGE desc-gen).
MODE = os.environ.get("KMODE", "gather")


def _build_program(n_tbl, d, t_tiles, chunk, mode=None):
    """Build the SPMD Bass program (identical across cores; data differs)."""
    from contextlib import ExitStack

    import concourse.bacc as bacc
    import concourse.mybir as mybir
    import concourse.tile as tile

    mode = mode or MODE
    dt = mybir.dt
    nc = bacc.Bacc(None, num_swdge_queues=4)

    idx_cols = (t_tiles * P) // 16
    if mode == "gather":
        wt = nc.declare_dram_parameter("wt", [n_tbl, d], dt.bfloat16, isOutput=False)
        idx_p = nc.declare_dram_parameter(
            "idx", [P, idx_cols], dt.int16, isOutput=False
        )
    else:
        wt = nc.declare_dram_parameter(
            "wt", [P, t_tiles, d], dt.bfloat16, isOutput=False
        )
    dest_p = nc.declare_dram_parameter(
        "dest", [P, t_tiles], dt.bfloat16, isOutput=False
    )
    iota_p = nc.declare_dram_parameter("iota", [P, P], dt.bfloat16, isOutput=False)
    out_p = nc.declare_dram_parameter(
        "out", [P, t_tiles, d], dt.bfloat16, isOutput=True
    )

    with tile.TileContext(nc) as tc:
        with ExitStack() as ctx:
            cpool = ctx.enter_context(tc.tile_pool(name="const", bufs=1))
            gpool = ctx.enter_context(tc.tile_pool(name="g", bufs=5))
            spool = ctx.enter_context(tc.tile_pool(name="s", bufs=8))
            opool = ctx.enter_context(tc.tile_pool(name="o", bufs=3))
            pspool = ctx.enter_context(tc.tile_pool(name="ps", bufs=8, space="PSUM"))

            if mode == "gather":
                idx_sb = cpool.tile([P, idx_cols], dtype=dt.int16)
                nc.sync.dma_start(idx_sb[:], idx_p[:])
            iota_sb = cpool.tile([P, P], dtype=dt.bfloat16)
            nc.sync.dma_start(iota_sb[:], iota_p[:])
            dest_sb = cpool.tile([P, t_tiles], dtype=dt.bfloat16)
            nc.sync.dma_start(dest_sb[:], dest_p[:])

            for gi, g0 in enumerate(range(0, t_tiles, chunk)):
                k = min(chunk, t_tiles - g0)
                gt = gpool.tile([P, k, d], dtype=dt.bfloat16, tag="g")
                if mode == "gather":
                    nc.gpsimd.dma_gather(
                        out_ap=gt[:],
                        in_ap=wt[:],
                        idxs_ap=idx_sb[:, g0 * 8 : (g0 + k) * 8],
                        num_idxs=P * k,
                        num_idxs_reg=P * k,
                        elem_size=d,
                        # >64 descriptors per SDMA engine don't fit one
                        # packet; the packed-packet path DMA-aborts >1024.
                        single_packet=False,
                        # queue 0 desc-gen occupies the Pool engine; 1-3
                        # run concurrently off-engine. Dispatch async queues
                        # first so q0's on-engine gen doesn't delay them.
                        queue_num=(1, 2, 3, 0)[gi % 4],
                    )
                else:
                    # Activation-issued HWDGE: keeps the input stream off the
                    # SP ring so it never queues behind a blocked out-write.
                    nc.scalar.dma_start(gt[:], wt[:, g0 : g0 + k, :])
                ot = opool.tile([P, k, d], dtype=dt.bfloat16, tag="o")
                # selection matrices for up to 4 tiles per DVE op
                s_tiles = {}
                for j0 in range(0, k, 4):
                    m = min(4, k - j0)
                    t = g0 + j0
                    s = spool.tile([P, m, P], dtype=dt.bfloat16, tag="s")
                    nc.vector.tensor_tensor(
                        out=s[:],
                        in0=dest_sb[:, t : t + m].unsqueeze(2).to_broadcast([P, m, P]),
                        in1=iota_sb[:].unsqueeze(1).to_broadcast([P, m, P]),
                        op=mybir.AluOpType.is_equal,
                    )
                    for j in range(m):
                        s_tiles[j0 + j] = (s, j)
                # matmuls in pairs sharing one PSUM tile; one cast-copy per
                # pair, rotated across the three elementwise-capable engines
                for j0 in range(0, k, 2):
                    m = min(2, k - j0)
                    ps = pspool.tile([P, m, d], dtype=dt.float32)
                    for j in range(m):
                        s, sj = s_tiles[j0 + j]
                        nc.tensor.matmul(
                            out=ps[:, j, :],
                            lhsT=s[:, sj, :],
                            rhs=gt[:, j0 + j, :],
                            start=True,
                            stop=True,
                        )
                    # GPSIMD cannot read PSUM — rotate Vector/Scalar only
                    if (g0 + j0) // 2 % 2 == 0:
                        nc.vector.tensor_copy(out=ot[:, j0 : j0 + m, :], in_=ps[:])
                    else:
                        nc.scalar.copy(out=ot[:, j0 : j0 + m, :], in_=ps[:])
                nc.sync.dma_start(out_p[:, g0 : g0 + k, :], ot[:])

    nc.finalize()
    return nc


def _pack_bins_exact(rows, counts, nbins):
    """Best-fit pack rows into bins with <=128 slots AND <=128 rows each.

    Returns (bin_of_row, pos_of_row) or None if infeasible.
    """
    n = len(counts)
    if nbins * P < counts.sum() or counts.max() > P:
        return None
    nz = np.flatnonzero(counts)
    order = nz[np.argsort(-counts[nz], kind="stable")]
    bin_of_row = np.full(n, -1, np.int64)
    loads = np.zeros(nbins, np.int64)
    nrows = np.zeros(nbins, np.int64)
    heap = [(0, b) for b in range(nbins)]
    heapq.heapify(heap)
    for r in order.tolist():
        c = int(counts[r])
        while True:
            if not heap:
                return None
            load, b = heapq.heappop(heap)
            if load != loads[b] or nrows[b] >= P:
                continue  # stale entry or row-capacity full
            break
        if load + c > P:
            return None  # min-load bin can't fit -> nothing can
        bin_of_row[r] = b
        loads[b] += c
        nrows[b] += 1
        if loads[b] < P and nrows[b] < P:
            heapq.heappush(heap, (int(loads[b]), b))
    # zero-count rows fill the remaining row capacity anywhere
    zeros = np.flatnonzero(counts == 0)
    cap = P - nrows
    if cap.sum() < len(zeros):
        return None
    fill_bins = np.repeat(np.arange(nbins), cap)[: len(zeros)]
    bin_of_row[zeros] = fill_bins
    # positions: stable order within bin
    order_all = np.argsort(bin_of_row, kind="stable")
    bins_sorted = bin_of_row[order_all]
    starts = np.searchsorted(bins_sorted, np.arange(nbins))
    pos_of_row = np.empty(n, np.int64)
    pos_of_row[order_all] = np.arange(n, dtype=np.int64) - starts[bins_sorted]
    if pos_of_row.max() >= P:
        return None
    return bin_of_row, pos_of_row


def _prepare(adj, weight):
    """Host-side sharding: pack rows into bins, build per-core gather data."""
    w = np.ascontiguousarray(np.asarray(weight, dtype=np.float32))
    n, d = w.shape
    adj = np.asarray(adj)
    rows = adj[0].astype(np.int64)
    cols = adj[1].astype(np.int64)

    counts = np.bincount(rows, minlength=n)
    t_tiles = T_TILES
    while True:
        nbins = NC_CORES * t_tiles
        packed = _pack_bins_exact(rows, counts, nbins)
        if packed is not None:
            break
        t_tiles += 2  # more slack; terminates long before degree bound bites
    bin_of_row, pos_of_row = packed

    # Edge slots: edges of a bin occupy consecutive slots ordered by source
    # column (ascending table reads within each tile chunk).
    eb = bin_of_row[rows]
    eo = np.lexsort((cols, eb))
    sb = eb[eo]
    starts = np.searchsorted(sb, np.arange(nbins))
    slot_in_bin = np.arange(len(eo), dtype=np.int64) - starts[sb]

    w_bf = w.astype(ml_dtypes.bfloat16)
    slots = t_tiles * P

    per_core = []
    for c in range(NC_CORES):
        sel = (sb // t_tiles) == c
        cols_c = cols[eo[sel]]
        gslot = (sb[sel] % t_tiles) * P + slot_in_bin[sel]
        uniq, inv = np.unique(cols_c, return_inverse=True)
        assert len(uniq) < 32768, len(uniq)
        idx_flat = np.zeros(slots, np.int16)
        dest_flat = np.full(slots, -1.0, np.float32)
        idx_flat[gslot] = inv.astype(np.int16)
        dest_flat[gslot] = pos_of_row[rows[eo[sel]]].astype(np.float32)
        col_flat = np.zeros(slots, np.int64)
        col_flat[gslot] = cols_c
        per_core.append((uniq, idx_flat, dest_flat, col_flat))

    n_tbl = max(len(u) for u, _, _, _ in per_core)
    n_tbl = -P * (-n_tbl // P)  # round up to multiple of 128

    iota = np.ascontiguousarray(
        np.broadcast_to(np.arange(P).astype(ml_dtypes.bfloat16), (P, P))
    )
    in_maps = []
    for uniq, idx_flat, dest_flat, col_flat in per_core:
        if MODE == "gather":
            tbl = np.zeros((n_tbl, d), ml_dtypes.bfloat16)
            tbl[: len(uniq)] = w_bf[uniq]
        else:
            # slot-ordered rows, partition-major: tbl[p, t, :] = row of
            # slot t*128+p (the layout dma_gather would produce).
            tbl = np.ascontiguousarray(
                w_bf[col_flat].reshape(t_tiles, P, d).transpose(1, 0, 2)
            )
        idx_wrapped = np.ascontiguousarray(idx_flat.reshape(-1, 16).T)  # [16, slots/16]
        idx_full = np.ascontiguousarray(np.tile(idx_wrapped, (8, 1)))  # [128, ...]
        dest_arr = np.ascontiguousarray(
            dest_flat.reshape(t_tiles, P).T.astype(ml_dtypes.bfloat16)
        )  # [128, T]
        m = {"wt": tbl, "dest": dest_arr, "iota": iota}
        if MODE == "gather":
            m["idx"] = idx_full
        in_maps.append(m)

    meta = {
        "n": n,
        "d": d,
        "t_tiles": t_tiles,
        "n_tbl": n_tbl,
        "bin_of_row": bin_of_row,
        "pos_of_row": pos_of_row,
    }
    return in_maps, meta


LAST_RESULT = None


def kernel(adj, size, weight):
    global LAST_RESULT
    from concourse.bass_utils import run_bass_kernel_spmd

    in_maps, meta = _prepare(adj, weight)
    nc = _build_program(meta["n_tbl"], meta["d"], meta["t_tiles"], CHUNK)
    res = run_bass_kernel_spmd(nc, in_maps, core_ids=list(range(NC_CORES)))
    LAST_RESULT = res
    t_tiles = meta["t_tiles"]
    # stack: [core, 128, T, d] -> index rows by (core, pos, local_tile)
    big = np.stack([np.asarray(r["out"]) for r in res.results])
    b = meta["bin_of_row"]
    out = big[b // t_tiles, meta["pos_of_row"], b % t_tiles, :]
    return np.ascontiguousarray(out.astype(np.float32))
